# revision 15
# baseline (speedup 1.0000x reference)
"""MoSARA MoE-routing kernel for 8 Trainium2 NeuronCores.

Math: the reference materializes per-expert delta weights
    delta_W[e] = U_k @ diag(lambda_k[e]) @ V_k,  out = sum_e g[b,e] * x @ (W+delta_W[e]).T
but since softmax gates sum to 1 this collapses to
    out = (x @ W.T + ((x @ V_k.T) * (g @ lambda_k)) @ U_k.T) * (1+v)
with g = softmax_e((x @ U_k @ router_W1) * router_W2[e]).

Host-side preprocessing (all exact, fp32):
  - fold (1+v) into W and U_k rows,
  - precompute u1 = U_k @ router_W1 (rank-1 router),
  - pre-transpose operands so the contraction dim lands on SBUF partitions,
  - main path in bf16; the low-rank path (a ~1% correction) in fp8e4m3 with
    power-of-2 scaling (V,U pre-scaled by 32; undone exactly at combine).

Device per core (data-parallel over B, 512 tokens/core):
  s1 = u1.T @ xT                    (1,512)  router logit, bf16
  sT = (32V).T-chunks @ xT8         (512,512) fp8 DoubleRow, 2x rate
  logits[e,b] = W2[e]*s1[b] - m[b]; m = exact row max via min of 2 scalings
  g = exp(logits); den = ones @ g; gn = g * bcast(1/den)  (approx recip)
  LamT = lam-chunks.T @ gn;  z = sT * LamT  stored fp8 (holds 32*z)
  W-pass:  psum = sum_d xT.T @ Wt   (bf16), evicted to SBUF fp32
  U-pass:  psum = z @ (32*Ut)       (fp8 DoubleRow), combined as
           out += psum * 2^-10      (exact power-of-2 unscale)
"""

import numpy as np
import ml_dtypes

import concourse.mybir as mybir
import concourse.tile as tile
from concourse import bacc
from concourse.bass_utils import run_bass_kernel_spmd

B, D, K, E = 4096, 2048, 512, 8
N_CORES = 8
BS = B // N_CORES          # 512 tokens per core
P = 128
ND = D // P                # 16 d-chunks
NC2 = ND // 2              # 8 DoubleRow d-chunk-pairs
NK = K // P                # 4 k-chunks
NN = D // 512              # 4 n-chunks of 512
NB = BS // P               # 4 b-chunks per core

BF16 = mybir.dt.bfloat16
F32 = mybir.dt.float32
FP8 = mybir.dt.float8e4
DR = mybir.MatmulPerfMode.DoubleRow
USCALE = 1.0 / 1024.0      # undo the 32x on V/U (32*32, exact power of 2)

_PROG = None


def _emit(tc, nc, xtd, xv8d, vt8d, wtd, ut8d, u1d, lamd, w2cd, nabd, outd):
    from contextlib import ExitStack

    with ExitStack() as ctx:
        const = ctx.enter_context(tc.tile_pool(name="const", bufs=1))
        xpool = ctx.enter_context(tc.tile_pool(name="xpool", bufs=1))
        wpool = ctx.enter_context(tc.tile_pool(name="wpool", bufs=1))
        work = ctx.enter_context(tc.tile_pool(name="work", bufs=1))
        opool = ctx.enter_context(tc.tile_pool(name="opool", bufs=1))
        ps = ctx.enter_context(tc.tile_pool(name="ps", bufs=8, space="PSUM"))

        # small constants (gpsimd SWDGE queue, except u1 which leads sync)
        u1_sb = const.tile([P, ND], BF16, tag="u1")
        nc.sync.dma_start(out=u1_sb[:], in_=u1d[:])
        lam_sb = const.tile([E, K], BF16, tag="lam")
        nc.gpsimd.dma_start(out=lam_sb[:], in_=lamd[:])
        w2c_sb = const.tile([1, E], BF16, tag="w2c")
        nc.gpsimd.dma_start(out=w2c_sb[:], in_=w2cd[:])
        nab_sb = const.tile([1, 2], F32, tag="nab")
        nc.gpsimd.dma_start(out=nab_sb[:], in_=nabd[:])
        ones8 = const.tile([E, 1], BF16, tag="ones8")
        nc.vector.memset(ones8[:], 1.0)
        ones18 = const.tile([1, E], BF16, tag="ones18")
        nc.vector.memset(ones18[:], 1.0)
        ones18f = const.tile([1, E], F32, tag="ones18f")
        nc.vector.memset(ones18f[:], 1.0)

        # streamed inputs on sync HWDGE in consumption order: per c-pair the
        # fp8 x/V chunks (phase 1) plus the bf16 x chunks (W-pass), then W.T,
        # then fp8 U.T
        xv8_sb = xpool.tile([P, ND, BS], FP8, tag="xv8")
        vt8_sb = xpool.tile([P, ND, K], FP8, tag="vt8")
        xts = [xpool.tile([P, BS], BF16, tag=f"xt{dc}", name=f"xt{dc}")
               for dc in range(ND)]
        for c in range(NC2):
            nc.sync.dma_start(out=xv8_sb[:, 2 * c:2 * c + 2, :],
                              in_=xv8d[:, 2 * c:2 * c + 2, :])
            nc.sync.dma_start(out=vt8_sb[:, 2 * c:2 * c + 2, :],
                              in_=vt8d[:, 2 * c:2 * c + 2, :])
            nc.sync.dma_start(out=xts[2 * c][:], in_=xtd[2 * c * P:(2 * c + 1) * P, :])
            nc.sync.dma_start(out=xts[2 * c + 1][:],
                              in_=xtd[(2 * c + 1) * P:(2 * c + 2) * P, :])
        wts = []
        for dc in range(ND):
            t = wpool.tile([P, D], BF16, tag=f"wt{dc}", name=f"wt{dc}")
            nc.sync.dma_start(out=t[:], in_=wtd[dc * P:(dc + 1) * P, :])
            wts.append(t)
        ut8_sb = wpool.tile([P, NK, D], FP8, tag="ut8")
        for kc in range(NK):
            nc.sync.dma_start(out=ut8_sb[:, kc:kc + 1, :], in_=ut8d[:, kc:kc + 1, :])

        # ---- phase 1 (c-pair-major, paced by DMA): sT fp8 DoubleRow + s1 ----
        s1_ps = ps.tile([1, BS], F32, tag="ps", name="s1_ps")
        sps = [ps.tile([P, BS], F32, tag="ps", name=f"sp{kc}") for kc in range(NK)]
        for c in range(NC2):
            for kc in range(NK):
                nc.tensor.matmul(sps[kc][:],
                                 vt8_sb[:, 2 * c:2 * c + 2, kc * P:(kc + 1) * P],
                                 xv8_sb[:, 2 * c:2 * c + 2, :],
                                 start=(c == 0), stop=(c == NC2 - 1),
                                 perf_mode=DR)
            for j in range(2):
                dc = 2 * c + j
                nc.tensor.matmul(s1_ps[:], u1_sb[:, dc:dc + 1], xts[dc][:],
                                 start=(dc == 0), stop=(dc == ND - 1))

        # -m[b] = min(-a*s1, -b*s1), a=max(W2), b=min(W2): exact row max shift
        s1row = work.tile([1, BS], BF16, tag="s1row")
        mneg = work.tile([1, BS], BF16, tag="mneg")
        ta = work.tile([1, BS], F32, tag="ta")
        tb = work.tile([1, BS], F32, tag="tb")
        nc.vector.tensor_copy(s1row[:], s1_ps[:])
        nc.vector.tensor_scalar_mul(ta[:], s1_ps[:], nab_sb[:, 0:1])
        nc.vector.tensor_scalar_mul(tb[:], s1_ps[:], nab_sb[:, 1:2])
        nc.vector.tensor_tensor(mneg[:], ta[:], tb[:], mybir.AluOpType.min)
        s_sb = []
        for kc in range(NK):
            t = work.tile([P, BS], F32, tag=f"s{kc}", name=f"s{kc}")
            nc.vector.tensor_copy(t[:], sps[kc][:])
            s_sb.append(t)

        # gating staging
        g_sb = work.tile([E, BS], BF16, tag="g")
        rden = work.tile([1, BS], F32, tag="rden")
        gn_sb = work.tile([E, BS], BF16, tag="gn")
        z3 = work.tile([P, NK, BS], FP8, tag="z3")

        def emit_gate_mm(step, pstate):
            if step == 0:
                e_ps = ps.tile([E, BS], F32, tag="ps", name="e_ps")
                nc.tensor.matmul(e_ps[:], w2c_sb[:], s1row[:], start=True, stop=False)
                nc.tensor.matmul(e_ps[:], ones18[:], mneg[:], start=False, stop=True)
                pstate["e_ps"] = e_ps
            elif step == 1:
                nc.scalar.activation(g_sb[:], pstate["e_ps"][:],
                                     mybir.ActivationFunctionType.Exp)
            elif step == 2:
                den_ps = ps.tile([1, BS], F32, tag="ps", name="den_ps")
                nc.tensor.matmul(den_ps[:], ones8[:], g_sb[:], start=True, stop=True)
                pstate["den_ps"] = den_ps
            elif step == 3:
                rden_f = work.tile([1, BS], F32, tag="rden_f")
                nc.vector.tensor_copy(rden_f[:], pstate["den_ps"][:])
                nc.vector.reciprocal_approx_fast(out=rden[:], in_=rden_f[:])
            elif step == 4:
                r8_ps = ps.tile([E, BS], F32, tag="ps", name="r8_ps")
                nc.tensor.matmul(r8_ps[:], ones18f[:], rden[:], start=True, stop=True)
                pstate["r8_ps"] = r8_ps
            elif step == 5:
                nc.vector.tensor_tensor(gn_sb[:], g_sb[:], pstate["r8_ps"][:],
                                        mybir.AluOpType.mult)

        def emit_lam_z(kc, pstate):
            lp = ps.tile([P, BS], F32, tag="ps", name=f"lp{kc}")
            nc.tensor.matmul(lp[:], lam_sb[:, kc * P:(kc + 1) * P],
                             gn_sb[:], start=True, stop=True)
            nc.vector.tensor_tensor(z3[:, kc, :], s_sb[kc][:], lp[:],
                                    mybir.AluOpType.mult)

        # ---- W pass: psum = x @ W'.T per (bc, ni), evict to o_sb fp32 ----
        pstate = {}
        gate_at = {1: 0, 3: 1, 5: 2, 7: 3, 9: 4, 11: 5}
        lam_at = {12: 0, 13: 1, 14: 2, 15: 3}
        o_sbs = []
        for bc in range(NB):
            psums = [ps.tile([P, 512], F32, tag="ps", name=f"po{bc}_{i}")
                     for i in range(NN)]
            for dc in range(ND):
                lhs = xts[dc][:, bc * P:(bc + 1) * P]
                for ni in range(NN):
                    nc.tensor.matmul(psums[ni][:], lhs,
                                     wts[dc][:, ni * 512:(ni + 1) * 512],
                                     start=(dc == 0), stop=(dc == ND - 1))
                if bc == 0 and dc in gate_at:
                    emit_gate_mm(gate_at[dc], pstate)
                if bc == 0 and dc in lam_at:
                    emit_lam_z(lam_at[dc], pstate)
            o_sb = opool.tile([P, D], F32, tag=f"o{bc}", name=f"o{bc}")
            o_sbs.append(o_sb)
            for ni in range(NN):
                nc.vector.tensor_copy(o_sb[:, ni * 512:(ni + 1) * 512], psums[ni][:])

        # ---- U pass: psum = (32z) @ (32U').T  (fp8 DoubleRow), then
        # out = o_sb + psum/1024, DMA out per bc ----
        for bc in range(NB):
            o_sb = o_sbs[bc]
            for ni in range(NN):
                pu = ps.tile([P, 512], F32, tag="ps", name=f"pu{bc}_{ni}")
                for c in range(NK // 2):
                    nc.tensor.matmul(pu[:],
                                     z3[:, 2 * c:2 * c + 2, bc * P:(bc + 1) * P],
                                     ut8_sb[:, 2 * c:2 * c + 2,
                                            ni * 512:(ni + 1) * 512],
                                     start=(c == 0), stop=(c == NK // 2 - 1),
                                     perf_mode=DR)
                nc.vector.scalar_tensor_tensor(
                    o_sb[:, ni * 512:(ni + 1) * 512], pu[:], USCALE,
                    o_sb[:, ni * 512:(ni + 1) * 512],
                    mybir.AluOpType.mult, mybir.AluOpType.add)
            nc.scalar.dma_start(out=outd[bc * P:(bc + 1) * P, :], in_=o_sb[:])


def build_program():
    nc = bacc.Bacc("TRN2", target_bir_lowering=False, debug=False)
    xtd = nc.dram_tensor("xt", (D, BS), BF16, kind="ExternalInput").ap()
    xv8d = nc.dram_tensor("xv8", (P, ND, BS), FP8, kind="ExternalInput").ap()
    vt8d = nc.dram_tensor("vt8", (P, ND, K), FP8, kind="ExternalInput").ap()
    wtd = nc.dram_tensor("wt", (D, D), BF16, kind="ExternalInput").ap()
    ut8d = nc.dram_tensor("ut8", (P, NK, D), FP8, kind="ExternalInput").ap()
    u1d = nc.dram_tensor("u1", (P, ND), BF16, kind="ExternalInput").ap()
    lamd = nc.dram_tensor("lam", (E, K), BF16, kind="ExternalInput").ap()
    w2cd = nc.dram_tensor("w2c", (1, E), BF16, kind="ExternalInput").ap()
    nabd = nc.dram_tensor("nab", (1, 2), F32, kind="ExternalInput").ap()
    outd = nc.dram_tensor("out", (BS, D), F32, kind="ExternalOutput").ap()

    with tile.TileContext(nc) as tc:
        _emit(tc, nc, xtd, xv8d, vt8d, wtd, ut8d, u1d, lamd, w2cd, nabd, outd)
    nc.compile()
    return nc


def _get_prog():
    global _PROG
    if _PROG is None:
        _PROG = build_program()
    return _PROG


def make_in_maps(x, W, U_k, V_k, lambda_k, v, router_W1, router_W2):
    bf = ml_dtypes.bfloat16
    f8 = mybir.dt.np(FP8)
    x = np.asarray(x, dtype=np.float32)
    W = np.asarray(W, dtype=np.float32)
    U_k = np.asarray(U_k, dtype=np.float32)
    V_k = np.asarray(V_k, dtype=np.float32)
    lambda_k = np.asarray(lambda_k, dtype=np.float32)
    v = np.asarray(v, dtype=np.float32)
    router_W1 = np.asarray(router_W1, dtype=np.float32)
    router_W2 = np.asarray(router_W2, dtype=np.float32)

    scale = 1.0 + v                                       # (D,) per output row n
    wt = np.ascontiguousarray((W * scale[:, None]).T).astype(bf)     # (d, n)
    # fp8 low-rank factors, pre-scaled by 32 (exact unscale on device)
    ut = (U_k * scale[:, None]).T * 32.0                             # (k, n)
    ut8 = np.ascontiguousarray(
        ut.reshape(NK, P, D).transpose(1, 0, 2)).astype(f8)          # (P, NK, D)
    vt = V_k.T * 32.0                                                # (d, k)
    vt8 = np.ascontiguousarray(
        vt.reshape(ND, P, K).transpose(1, 0, 2)).astype(f8)          # (P, ND, K)
    u1 = (U_k.astype(np.float64) @ router_W1.astype(np.float64)).astype(np.float32)
    u1 = np.ascontiguousarray(u1.reshape(ND, P).T).astype(bf)        # (P, ND)
    lam = np.ascontiguousarray(lambda_k).astype(bf)                  # (E, K)
    w2 = router_W2.reshape(-1)
    w2c = np.ascontiguousarray(w2.reshape(1, E)).astype(bf)
    nab = np.array([[-w2.max(), -w2.min()]], dtype=np.float32)

    in_maps = []
    for c in range(N_CORES):
        xT = x[c * BS:(c + 1) * BS].T                                # (D, BS)
        xt = np.ascontiguousarray(xT).astype(bf)
        xv8 = np.ascontiguousarray(
            xT.reshape(ND, P, BS).transpose(1, 0, 2)).astype(f8)     # (P, ND, BS)
        in_maps.append({"xt": xt, "xv8": xv8, "vt8": vt8, "wt": wt, "ut8": ut8,
                        "u1": u1, "lam": lam, "w2c": w2c, "nab": nab})
    return in_maps


def run(in_maps, trace=False):
    nc = _get_prog()
    res = run_bass_kernel_spmd(nc, in_maps, core_ids=list(range(N_CORES)), trace=trace)
    out = np.concatenate([res.results[c]["out"] for c in range(N_CORES)], axis=0)
    return out, res


def kernel(x, W, U_k, V_k, lambda_k, v, router_W1, router_W2):
    in_maps = make_in_maps(x, W, U_k, V_k, lambda_k, v, router_W1, router_W2)
    out, _ = run(in_maps, trace=False)
    return out
